# revision 25
# baseline (speedup 1.0000x reference)
"""RNN-T JointNetwork kernel for 8x Trainium2 NeuronCores.

Sharding: data-parallel over batch (B=8 -> 1 batch element per core).
Each core computes its (T, U, V) logit block on-chip; host does the tiny
input projections (f32) and the final transpose + b_out.

bf16 out-projection at the PE roofline (~167us of matmul stream). fp8
DoubleRow gives 2x contraction per instruction but e4m3's quantization
error (3.6e-2 plain, 1.8e-2 with grid-dither pairs, measured) cannot
beat bf16's cycles/col under the 2e-2 gate; uint8 / e3m4-DoubleRow
matmuls are rejected by the walrus BIR verifier. So the wins here are
structural:
- 18 warmup matmuls on junk tiles from t~0 warm the PE through the DMA
  feed (HAM hits K=8/8 before the real stream; no cold matmuls).
- Superblock 0 runs its matmuls jc-OUTER with all 8 vc accumulation
  groups open across 8 single-bank psum tiles: the PE starts right
  after the first tanh chunk (~4.5us) instead of after all five
  (~13us), overlapping the scalar-engine tanh chain.
- encP is bf16 (halves the startup DMA feed; rel err 2.8e-3 -> 3.1e-3,
  gate is 2e-2).
- The joint grid is u-major flat (10000 cols, 10 superblocks of 1000);
  tanh(encP + predB[u]) fuses the per-u bias into one scalar-engine
  activation per (jc, u); 500-col matmuls h-inner per jc so LDWEIGHTS
  hides under the stream; vector evacuates PSUM->SBUF bf16; the last
  superblock splits evacuation across vector+scalar and DMAs per half
  to shorten the tail.

Output is [V, U*T] bf16; the host transposes back and adds b_out.
"""

import numpy as np
import ml_dtypes

P = 128
B, T, U = 8, 200, 50
DE, DP, DJ, V = 512, 640, 640, 1024
NJC, NVC = DJ // P, V // P  # 5, 8
USB = 5              # u's per superblock
CSB = USB * T        # 1000 joint positions per superblock
NSB = U // USB       # 10 superblocks
NH = 2               # 500-col matmul halves per superblock
CH = CSB // NH       # 500
NWARM = 9            # HAM warmup matmuls (fill preamble->first-real-MM gap)

BF16 = ml_dtypes.bfloat16

_module = None


def _build_module():
    import concourse.bass as bass
    import concourse.mybir as mybir
    import concourse.tile as tile
    from concourse import bacc

    bf = mybir.dt.bfloat16
    f32 = mybir.dt.float32
    Act = mybir.ActivationFunctionType
    ts, ds = bass.ts, bass.ds

    nc = bacc.Bacc("TRN2", target_bir_lowering=False, debug=False)

    d_encP = nc.dram_tensor("encP", (P, NJC, T), bf, kind="ExternalInput").ap()
    d_predB = nc.dram_tensor("predB", (P, NJC, U), f32, kind="ExternalInput").ap()
    d_woutT = nc.dram_tensor("woutT", (P, NJC, V), bf, kind="ExternalInput").ap()
    d_out = nc.dram_tensor("out", (V, U * T), bf, kind="ExternalOutput").ap()

    with tile.TileContext(nc) as tc:
        with (
            tc.tile_pool(name="consts", bufs=1) as consts,
            tc.tile_pool(name="joints", bufs=10) as joints,
            tc.tile_pool(name="outsb", bufs=8) as outsb,
            tc.tile_pool(name="ps", bufs=8, space="PSUM") as pspool,
        ):
            # input DMAs on the sync ring in first-use order
            predB = consts.tile([P, NJC, U], f32)
            encP = consts.tile([P, NJC, T], bf)
            wout = consts.tile([P, NJC, V], bf)
            # warmup tiles (never DMA'd; memset then matmul'd repeatedly)
            wjunk = consts.tile([P, 512], bf)
            wwt = consts.tile([P, P], bf)
            junk2 = consts.tile([P, 8], bf)
            nc.vector.memset(wjunk[:], 0.0)
            nc.vector.memset(wwt[:], 0.0)
            # dummy tanh: forces the ACT_TABLE_LOAD (1.3us) to run during
            # the preamble instead of after the first data arrives
            nc.scalar.activation(junk2[:], wjunk[:, :8], Act.Tanh)
            # single contiguous transfers (strided per-vc slices cost
            # ~650ns of ring serialization each and transfer slowly);
            # wout arrives in per-jc slices matching superblock 0's
            # jc-outer consumption order.
            nc.sync.dma_start(encP[:, 0], d_encP[:, 0])
            nc.sync.dma_start(predB[:], d_predB[:])
            nc.sync.dma_start(wout[:, 0], d_woutT[:, 0])
            nc.sync.dma_start(encP[:, 1:], d_encP[:, 1:])
            for jc in range(1, NJC):
                nc.sync.dma_start(wout[:, jc], d_woutT[:, jc])

            # HAM warmup: junk matmuls keep the PE busy from t~0 until the
            # first real matmuls (~4.3us); the psum tile frees just before
            # superblock 0 needs the 8th bank.
            pwarm = pspool.tile([P, CH], f32, tag="ps", name="pwarm")
            for _ in range(NWARM):
                nc.tensor.matmul(pwarm[:], wwt[:], wjunk[:, :CH], start=True, stop=True)

            def make_joint(sb):
                jflat = []
                for jc in range(NJC):
                    jt = joints.tile([P, USB, T], bf, tag="jt")
                    for i in range(USB):
                        nc.scalar.activation(
                            jt[:, i, :], encP[:, jc, :], Act.Tanh,
                            bias=predB[:, jc, sb * USB + i, None],
                        )
                    jflat.append(jt[:].rearrange("p a b -> p (a b)"))
                return jflat

            # --- superblock 0: jc-outer, 8 accumulation groups open at
            # once, so matmuls start as soon as tanh chunk 0 exists.
            jflat = make_joint(0)
            for h in range(NH):
                pss8 = [
                    pspool.tile([P, CH], f32, tag="ps", name=f"ps0_{h}_{vc}")
                    for vc in range(NVC)
                ]
                for jc in range(NJC):
                    for vc in range(NVC):
                        nc.tensor.matmul(
                            pss8[vc][:],
                            wout[:, jc, ts(vc, P)],
                            jflat[jc][:, ds(h * CH, CH)],
                            start=(jc == 0), stop=(jc == NJC - 1),
                        )
                for vc in range(NVC):
                    osb = outsb.tile([P, CH], bf, tag="osb", name=f"osb0_{h}_{vc}")
                    nc.vector.tensor_copy(osb[:], pss8[vc][:])
                    nc.sync.dma_start(
                        d_out[ds(vc * P, P), ds(h * CH, CH)], osb[:]
                    )

            # --- superblocks 1..9: vc-outer steady state
            for sb in range(1, NSB):
                jflat = make_joint(sb)
                for vc in range(NVC):
                    pss = [
                        pspool.tile([P, CH], f32, tag="ps", name=f"ps_o{h}")
                        for h in range(NH)
                    ]
                    for jc in range(NJC):
                        for h in range(NH):
                            mm = nc.tensor.matmul(
                                pss[h][:],
                                wout[:, jc, ts(vc, P)],
                                jflat[jc][:, ds(h * CH, CH)],
                                start=(jc == 0), stop=(jc == NJC - 1),
                            )
                            if h > 0:
                                # same stationary weights as h=0: skip the
                                # redundant LDWEIGHTS
                                mm.ldweights = False
                    osb = outsb.tile([P, CSB], bf, tag="osb")
                    if sb < NSB - 1:
                        for h in range(NH):
                            nc.vector.tensor_copy(osb[:, ds(h * CH, CH)], pss[h][:])
                        nc.sync.dma_start(d_out[ds(vc * P, P), ts(sb, CSB)], osb[:])
                    else:
                        # tail: h0 on vector + DMA; the final h1
                        # evacuation splits across vector+scalar so the
                        # last DMA issues ~250ns sooner
                        HH = CH // 2
                        nc.vector.tensor_copy(osb[:, ds(0, CH)], pss[0][:])
                        nc.sync.dma_start(
                            d_out[ds(vc * P, P), ds(sb * CSB, CH)],
                            osb[:, ds(0, CH)],
                        )
                        nc.vector.tensor_copy(
                            osb[:, ds(CH, HH)], pss[1][:, ds(0, HH)]
                        )
                        nc.scalar.copy(
                            osb[:, ds(CH + HH, HH)], pss[1][:, ds(HH, HH)]
                        )
                        # push the last DMA from the scalar ring so it
                        # doesn't serialize behind h0's push on sync
                        nc.scalar.dma_start(
                            d_out[ds(vc * P, P), ds(sb * CSB + CH, CH)],
                            osb[:, ds(CH, CH)],
                        )

    nc.compile()
    return nc


def _get_module():
    global _module
    if _module is None:
        _module = _build_module()
    return _module


def _chunk(x2d, dtype=BF16):
    """(n*128, C...) -> (128, n, C...) partition-chunked, contiguous."""
    n = x2d.shape[0] // P
    return np.ascontiguousarray(
        x2d.reshape((n, P) + x2d.shape[1:]).swapaxes(0, 1)
    ).astype(dtype)


def make_in_maps(encoder_out, predictor_out, W_enc, b_enc, W_pred, b_pred, W_out, b_out):
    woutT = _chunk(np.ascontiguousarray(W_out.T))       # (128, 5, 1024)
    # host-side projections, f32 (1% of total FLOPs)
    enc = np.einsum("btd,jd->bjt", encoder_out, W_enc)              # (B, 640, 200)
    pred = np.einsum("bud,jd->bju", predictor_out, W_pred)          # (B, 640, 50)
    pred += (b_enc + b_pred)[None, :, None]
    in_maps = []
    for b in range(B):
        in_maps.append({
            "encP": _chunk(enc[b], BF16),           # (128, 5, 200) bf16
            "predB": _chunk(pred[b], np.float32),   # (128, 5, 50) f32
            "woutT": woutT,
        })
    return in_maps


def _postprocess(out_vut, b_out):
    """(V, U*T) device output (bf16) -> (T, U, V) fp32 with vocab bias."""
    out = out_vut.astype(np.float32).reshape(V, U, T).T  # (T, U, V)
    return out + b_out.astype(np.float32)


def kernel(encoder_out, predictor_out, W_enc, b_enc, W_pred, b_pred, W_out, b_out):
    from concourse.bass_utils import run_bass_kernel_spmd

    nc = _get_module()
    in_maps = make_in_maps(
        encoder_out, predictor_out, W_enc, b_enc, W_pred, b_pred, W_out, b_out
    )
    res = run_bass_kernel_spmd(nc, in_maps, list(range(B)))
    out = np.empty((B, T, U, V), np.float32)
    for b in range(B):
        out[b] = _postprocess(res.results[b]["out"], b_out)
    return out
